# revision 30
# baseline (speedup 1.0000x reference)
"""Trainium2 Bass kernel for nn_AttentionLayer (pooling, dim=0 softmax).

Computation (full shapes B=64, T=2048, D=256):
    u = tanh(hs @ W^T + b)            [B,T,D]
    scores = u @ v                    [B,T]
    a = softmax(scores, axis=0)       (over the batch axis!)
    s[b] = a[b] @ hs[b]               [B,D]

Sharding: sequence-parallel over T across 8 cores (T_loc = 256). The
dim=0 softmax couples samples but not time steps, so each core's
softmax is fully local; only the final weighted sum needs a cross-core
reduction, done on the host (8 x 64KB partials).

The host pre-packs each core's shard into two fp16 layouts (natural
[t, d] for the weighted sum, d-major for the tanh matmul), with rows
permuted (c, b, q)-major (t_loc = c*128 + q, so tile j = c*64 + b is
one sample's half-window and softmax half c needs only groups
[8c, 8c+8)). Device pipeline per group (PE ~97% busy end to end,
fp16 compute / f32 PSUM, rel err ~3e-3):

  1. all 16 xt loads are triggered upfront on the sync queue in
     consumption order (FIFO queues then deliver them in the order the
     PE consumes them); xn loads are paced behind the compute pipeline
     via tiny WAW gate-writes so their traffic stays out of the
     startup window; ~24 identity-transpose spins ramp the PE p-state
     during the initial fill.
  2. PE mm1: z^T[e, bt] = W-chunk @ Xt-chunk (PSUM f32, N=512).
  3. ACT: u = tanh(z + bias), PSUM -> SBUF fp16, per-partition bias.
  4. PE mm2: score pieces [1, 512] = v^T @ u accumulate into a PSUM
     collector [97, 512] at partition rows {0, 32, 64, 96} (the only
     legal output bases for 1-row matmuls). One full-height DVE cast
     per group-pair moves 4 pieces to SBUF fp16 at once.
  5. PE transposes the four 128-col chunks of each cast pair straight
     into scmat[q, j] columns (~47ns each): scores never touch DRAM
     and never ride a DMA, so the softmax input has no completion-
     semaphore latency on the critical path.
  6. softmax over b per half c on contiguous scmat columns; no
     max-subtraction (|scores| < ~70 so exp fits f32 comfortably);
     exp is WAW-gated behind the last tanh so the Tile scheduler
     cannot hoist it into the in-order ACT stream mid-kernel.
     Normalized weights are written into a zero-initialized
     block-diagonal slab via a stride-65 access pattern.
  7. PE step4: 128 matmuls lhsT=slab[:, j, :] (zero-padded a-columns)
     accumulate one [64, 256] PSUM tile in a single accumulation
     group, starting immediately after the last mm2 (the c=0 slab
     half is ready mid-kernel; the c=1 softmax chain completes under
     the first 64 step4 matmuls).
  8. one PSUM->SBUF copy + DMA out s_partial [64, 256] f32; the host
     sums the 8 partials.
"""

import numpy as np

B, T, D = 64, 2048, 256
NCORES = 8
T_LOC = T // NCORES          # 256
BT = B * T_LOC               # 16384 rows per core
BTG = 1024                   # bt rows per pipeline group (2 PSUM banks)
PH = 128                     # partitions
XN_DEFER = 2                 # groups of xn-load deferral


def build_program(b_dim=B):
    import concourse.bacc as bacc
    import concourse.tile as tile
    from concourse import mybir

    F32 = mybir.dt.float32
    F16 = mybir.dt.float16
    AF = mybir.ActivationFunctionType

    assert b_dim == B
    bt = b_dim * T_LOC
    n_groups = bt // BTG          # 16
    tiles_per_g = BTG // PH       # 8
    n_cols = bt // PH             # 128 tiles j = c*64 + b

    nc = bacc.Bacc("TRN2", target_bir_lowering=False, debug=False)

    # Host-prepacked fp16 inputs; rows are permuted (c, b, q)-major:
    #   row = c*8192 + b*128 + q  with  t_loc = c*128 + q
    #   xn16[g, p, i, d]  = hsP[g*1024 + i*128 + p, d]          (natural)
    #   xt16[g, p, m, q]  = hsP[g*1024 + (m//2)*128 + q, (m%2)*128 + p]
    #   wt16[p, dc, ec, e'] = W[ec*128 + e', dc*128 + p]
    xn_d = nc.dram_tensor(
        "xn16", [n_groups, PH, tiles_per_g, D], F16, kind="ExternalInput"
    ).ap()
    xt_d = nc.dram_tensor(
        "xt16", [n_groups, PH, 2 * tiles_per_g, PH], F16, kind="ExternalInput"
    ).ap()
    wt_d = nc.dram_tensor("wt16", [PH, 2, 2, PH], F16, kind="ExternalInput").ap()
    bias_d = nc.dram_tensor("bias2", [PH, 2], F32, kind="ExternalInput").ap()
    v_d = nc.dram_tensor("v2", [PH, 2], F16, kind="ExternalInput").ap()
    out = nc.dram_tensor("out", [b_dim, D], F32, kind="ExternalOutput").ap()

    with tile.TileContext(nc) as tc:
        with (
            tc.tile_pool(name="singles", bufs=1) as singles,
            tc.tile_pool(name="xnat", bufs=n_groups) as xnat_pool,
            tc.tile_pool(name="xt", bufs=10) as xt_pool,
            tc.tile_pool(name="usb", bufs=4) as u_pool,
            tc.tile_pool(name="small", bufs=8) as small,
            tc.tile_pool(name="dram", bufs=1, space="DRAM") as dram_pool,
        ):
            # identity first: it unblocks the PE warmup spins
            identity16 = singles.tile([PH, PH], F16)
            from concourse.masks import make_identity
            make_identity(nc, identity16)

            # ---- constants on gpsimd so the sync queue leads with
            # xt0 (which gates the first matmul together with wt) ----
            wt = singles.tile([PH, 2, 2, PH], F16)
            nc.gpsimd.dma_start(out=wt, in_=wt_d)
            bias_sb = singles.tile([PH, 2], F32)
            nc.gpsimd.dma_start(out=bias_sb, in_=bias_d)
            v16 = singles.tile([PH, 2], F16)
            nc.gpsimd.dma_start(out=v16, in_=v_d)

            # transposed scores [q, j] (j = c*64 + b)
            scmat = singles.tile([PH, n_cols], F16)

            # block-diagonal slab for step4: slab[q, j, b'] nonzero only
            # at b' = j%64; zeroed per half on DVE early
            slab = singles.tile([PH, n_cols, b_dim], F16)
            slab_flat = slab.rearrange("p j b -> p (j b)")
            nc.vector.memset(slab_flat[:, 0:4096], 0.0)      # c=0 half
            # (c=1 half is zeroed inside the loop, off the cast path)

            s_sb = singles.tile([b_dim, D], F32)

            xnat_tiles = []

            def emit_xn_load(g, gate=None):
                xn = xnat_pool.tile([PH, tiles_per_g, D], F16, name="xn",
                                    bufs=16)
                if gate is not None:
                    # tiny WAW write: the load waits for it, pacing the
                    # trigger to the compute pipeline
                    nc.vector.tensor_copy(xn[0:1, 0:1, 0:1], gate[0:1, 0:1])
                nc.gpsimd.dma_start(out=xn, in_=xn_d[g])
                xnat_tiles.append(xn)

            def emit_softmax_half(c, gate=None):
                # no max-subtraction: |scores| < ~70, exp fits f32
                e_sb = small.tile([PH, 64], F32, name="esb", bufs=2)
                ssum = small.tile([PH, 1], F32, name="ssum", bufs=2)
                if gate is not None:
                    # WAW pin: stops the scheduler from hoisting this exp
                    # into the mid-kernel tanh stream (ACT is in-order)
                    nc.scalar.copy(e_sb[0:1, 0:1], gate[0:1, 0:1])
                nc.scalar.activation(
                    e_sb, scmat[:, c * 64:(c + 1) * 64], AF.Exp,
                    accum_out=ssum,
                )
                rec = small.tile([PH, 1], F32, name="rec", bufs=2)
                nc.vector.reciprocal(rec, ssum)
                # diagonal: j = c*64+b, flat off = j*64 + b = c*4096 + 65*b
                diag = slab_flat[:, c * 4096:c * 4096 + 63 * 65 + 1:65]
                nc.vector.tensor_scalar_mul(diag, e_sb, rec)

            # ---- PE p-state warmup spins during the DMA fill ----
            with tc.tile_pool(name="warm", bufs=1, space="PSUM") as warm_pool:
                warm_ps = warm_pool.tile([PH, PH], F16)
                for _ in range(24):
                    nc.tensor.transpose(warm_ps, identity16, identity16)

            with (
                tc.tile_pool(name="ups", bufs=3, space="PSUM") as ups_pool,
                tc.tile_pool(name="coll", bufs=1, space="PSUM") as coll_pool,
                tc.tile_pool(name="trp", bufs=1, space="PSUM") as trp_pool,
            ):
                coll = None
                tr_ps = trp_pool.tile([PH, 4, PH], F16)

                xt_tiles = [
                    xt_pool.tile([PH, 2 * tiles_per_g, PH], F16, name="xt",
                                 bufs=n_groups)
                    for _ in range(n_groups)
                ]
                # all xt triggers upfront on sync in consumption order:
                # FIFO queues then deliver groups in the order the PE
                # consumes them, with no cross-engine coupling
                nc.sync.dma_start(
                    out=xt_tiles[0][:, 0:tiles_per_g, :],
                    in_=xt_d[0, :, 0:tiles_per_g, :],
                )
                nc.sync.dma_start(
                    out=xt_tiles[0][:, tiles_per_g:, :],
                    in_=xt_d[0, :, tiles_per_g:, :],
                )
                for g in range(1, n_groups):
                    nc.sync.dma_start(out=xt_tiles[g], in_=xt_d[g])

                for g in range(n_groups):
                    xt = xt_tiles[g]

                    # ---- mm1 + tanh ----
                    u16_cur = []
                    for ec in range(2):
                        u_ps = ups_pool.tile([PH, BTG], F32, name="ups",
                                             bufs=3)
                        for half in range(2):
                            for dc in range(2):
                                m0 = half * 8 + dc
                                nc.tensor.matmul(
                                    u_ps[:, half * 512:(half + 1) * 512],
                                    wt[:, dc, ec, :],
                                    xt[:, m0:m0 + 7:2, :],
                                    start=(dc == 0),
                                    stop=(dc == 1),
                                )
                        u_sb = u_pool.tile([PH, BTG], F16, name="usb", bufs=4)
                        nc.scalar.activation(
                            u_sb, u_ps, AF.Tanh, bias=bias_sb[:, ec:ec + 1]
                        )
                        u16_cur.append(u_sb)

                    if g >= XN_DEFER:
                        emit_xn_load(g - XN_DEFER, gate=u16_cur[0])

                    if g == n_groups - 1:
                        # fill the PE wait on the last tanh with spins
                        for _ in range(6):
                            nc.tensor.transpose(
                                tr_ps[:, 0, :], identity16, identity16
                            )

                    # ---- mm2 into the PSUM collector: piece (g, half)
                    # at partition 32*((g%2)*2 + half) ----
                    if g % 2 == 0:
                        coll = coll_pool.tile([97, 512], F32)
                    for half in range(2):
                        row = 32 * ((g % 2) * 2 + half)
                        for ec in range(2):
                            nc.tensor.matmul(
                                coll[row:row + 1, :],
                                v16[:, ec:ec + 1],
                                u16_cur[ec][:, half * 512:(half + 1) * 512],
                                start=(ec == 0),
                                stop=(ec == 1),
                                tile_position=(0, row),
                            )
                    if g % 2 == 1:
                        # cast 4 pieces at once, then PE-transpose the
                        # four 128-column chunks straight into scmat
                        # columns (no DRAM bounce, no DMA-transpose:
                        # scores stay in SBUF/PSUM end to end). Chunk c
                        # row 32r holds scmat column j = 16p + 4r + c.
                        p = g // 2
                        sc16 = small.tile([97, 512], F16, name="sc16", bufs=2)
                        nc.vector.tensor_copy(sc16, coll)
                        for c in range(4):
                            nc.tensor.transpose(
                                tr_ps[:, c, 0:97],
                                sc16[:, c * PH:(c + 1) * PH],
                                identity16[0:97, 0:97],
                            )
                        nc.vector.tensor_copy(
                            scmat[:, p * 16:(p + 1) * 16]
                            .rearrange("p (r c) -> p c r", c=4),
                            tr_ps[:, :, 0:97:32],
                        )
                    if g == 2:
                        nc.vector.memset(slab_flat[:, 4096:8192], 0.0)

            # ---- tail ----
            with tc.tile_pool(name="s4ps", bufs=1, space="PSUM") as s4_pool:
                s_ps = s4_pool.tile([b_dim, D], F32)
                for g in range(n_groups - XN_DEFER, n_groups):
                    emit_xn_load(g)
                emit_softmax_half(0, gate=u16_cur[0])
                emit_softmax_half(1, gate=u16_cur[1])
                for j in range(n_cols):
                    nc.tensor.matmul(
                        s_ps,
                        slab[:, j, :],
                        xnat_tiles[j // tiles_per_g][:, j % tiles_per_g, :],
                        start=(j == 0),
                        stop=(j == n_cols - 1),
                    )
                nc.vector.tensor_copy(s_sb, s_ps)
                nc.sync.dma_start(out=out, in_=s_sb)

    nc.compile()
    return nc


_prog_cache = {}


def _get_program(b_dim):
    if b_dim not in _prog_cache:
        _prog_cache[b_dim] = build_program(b_dim)
    return _prog_cache[b_dim]


def prep_core_inputs(shard_f32, w, bias, v):
    """Pack one core's [bt, D] f32 shard + weights into device layouts.

    Rows are permuted (c, b, q)-major first: rowP = c*8192 + b*128 + q
    for original row b*256 + c*128 + q.
    """
    bt = shard_f32.shape[0]
    b_dim = bt // T_LOC
    n_groups = bt // BTG
    tiles_per_g = BTG // PH
    h16 = shard_f32.astype(np.float16)
    h16 = np.ascontiguousarray(
        h16.reshape(b_dim, 2, PH, D).transpose(1, 0, 2, 3)
    ).reshape(bt, D)
    # xn16[g, p, i, d] = h16[g*1024 + i*128 + p, d]
    xn16 = np.ascontiguousarray(
        h16.reshape(n_groups, tiles_per_g, PH, D).transpose(0, 2, 1, 3)
    )
    # xt16[g, p, m, q] = h16[g*1024 + (m//2)*128 + q, (m%2)*128 + p]
    hr = h16.reshape(n_groups, tiles_per_g, PH, 2, PH)
    xt16 = np.ascontiguousarray(hr.transpose(0, 4, 1, 3, 2)).reshape(
        n_groups, PH, 2 * tiles_per_g, PH
    )
    # wt16[p, dc, ec, e'] = W[ec*128 + e', dc*128 + p]
    w16 = w.astype(np.float16)
    wt16 = np.ascontiguousarray(
        w16.reshape(2, PH, 2, PH).transpose(3, 2, 0, 1)
    )
    bias2 = np.ascontiguousarray(bias.reshape(2, PH).T).astype(np.float32)
    v2 = np.ascontiguousarray(v.reshape(2, PH).T).astype(np.float16)
    return {
        "xn16": xn16,
        "xt16": xt16,
        "wt16": wt16,
        "bias2": bias2,
        "v2": v2,
    }


def kernel(hidden_states, W_attention, bias_attention, attention_vector):
    from concourse.bass_utils import run_bass_kernel_spmd

    hs = np.asarray(hidden_states, dtype=np.float32)
    w = np.asarray(W_attention, dtype=np.float32)
    bias = np.asarray(bias_attention, dtype=np.float32)
    v = np.asarray(attention_vector, dtype=np.float32)

    nc = _get_program(B)

    in_maps = []
    for core in range(NCORES):
        shard = np.ascontiguousarray(
            hs[:, core * T_LOC:(core + 1) * T_LOC, :]
        ).reshape(BT, D)
        in_maps.append(prep_core_inputs(shard, w, bias, v))

    res = run_bass_kernel_spmd(nc, in_maps, list(range(NCORES)))
    s = np.zeros((B, D), dtype=np.float32)
    for r in res.results:
        s += r["out"]
    return s


# revision 31
# speedup vs baseline: 1.0137x; 1.0137x over previous
"""Trainium2 Bass kernel for nn_AttentionLayer (pooling, dim=0 softmax).

Computation (full shapes B=64, T=2048, D=256):
    u = tanh(hs @ W^T + b)            [B,T,D]
    scores = u @ v                    [B,T]
    a = softmax(scores, axis=0)       (over the batch axis!)
    s[b] = a[b] @ hs[b]               [B,D]

Sharding: sequence-parallel over T across 8 cores (T_loc = 256). The
dim=0 softmax couples samples but not time steps, so each core's
softmax is fully local; only the final weighted sum needs a cross-core
reduction, done on the host (8 x 64KB partials).

The host pre-packs each core's shard into two fp16 layouts (natural
[t, d] for the weighted sum, d-major for the tanh matmul), with rows
permuted (c, b, q)-major (t_loc = c*128 + q, so tile j = c*64 + b is
one sample's half-window and softmax half c needs only groups
[8c, 8c+8)). Device pipeline per group (PE ~97% busy end to end,
fp16 compute / f32 PSUM, rel err ~3e-3):

  1. all 16 xt loads are triggered upfront on the sync queue in
     consumption order (FIFO queues then deliver them in the order the
     PE consumes them); xn loads are paced behind the compute pipeline
     via tiny WAW gate-writes so their traffic stays out of the
     startup window; ~24 identity-transpose spins ramp the PE p-state
     during the initial fill.
  2. PE mm1: z^T[e, bt] = W-chunk @ Xt-chunk (PSUM f32, N=512).
  3. ACT: u = tanh(z + bias), PSUM -> SBUF fp16, per-partition bias.
  4. PE mm2: score pieces [1, 512] = v^T @ u accumulate into a PSUM
     collector [97, 512] at partition rows {0, 32, 64, 96} (the only
     legal output bases for 1-row matmuls). One full-height DVE cast
     per group-pair moves 4 pieces to SBUF fp16 at once.
  5. PE transposes the four 128-col chunks of each cast pair straight
     into scmat[q, j] columns (~47ns each): scores never touch DRAM
     and never ride a DMA, so the softmax input has no completion-
     semaphore latency on the critical path.
  6. softmax over b per half c on contiguous scmat columns; no
     max-subtraction (|scores| < ~70 so exp fits f32 comfortably);
     exp is WAW-gated behind the last tanh so the Tile scheduler
     cannot hoist it into the in-order ACT stream mid-kernel.
     Normalized weights are written into a zero-initialized
     block-diagonal slab via a stride-65 access pattern.
  7. PE step4: 128 matmuls lhsT=slab[:, j, :] (zero-padded a-columns)
     accumulate one [64, 256] PSUM tile in a single accumulation
     group, starting immediately after the last mm2 (the c=0 slab
     half is ready mid-kernel; the c=1 softmax chain completes under
     the first 64 step4 matmuls).
  8. one PSUM->SBUF copy + DMA out s_partial [64, 256] f32; the host
     sums the 8 partials.
"""

import numpy as np

B, T, D = 64, 2048, 256
NCORES = 8
T_LOC = T // NCORES          # 256
BT = B * T_LOC               # 16384 rows per core
BTG = 1024                   # bt rows per pipeline group (2 PSUM banks)
PH = 128                     # partitions
XN_DEFER = 2                 # groups of xn-load deferral


def build_program(b_dim=B):
    import concourse.bacc as bacc
    import concourse.tile as tile
    from concourse import mybir

    F32 = mybir.dt.float32
    F16 = mybir.dt.float16
    AF = mybir.ActivationFunctionType

    assert b_dim == B
    bt = b_dim * T_LOC
    n_groups = bt // BTG          # 16
    tiles_per_g = BTG // PH       # 8
    n_cols = bt // PH             # 128 tiles j = c*64 + b

    nc = bacc.Bacc("TRN2", target_bir_lowering=False, debug=False)

    # Host-prepacked fp16 inputs; rows are permuted (c, b, q)-major:
    #   row = c*8192 + b*128 + q  with  t_loc = c*128 + q
    #   xn16[g, p, i, d]  = hsP[g*1024 + i*128 + p, d]          (natural)
    #   xt16[g, p, m, q]  = hsP[g*1024 + (m//2)*128 + q, (m%2)*128 + p]
    #   wt16[p, dc, ec, e'] = W[ec*128 + e', dc*128 + p]
    xn_d = nc.dram_tensor(
        "xn16", [n_groups, PH, tiles_per_g, D], F16, kind="ExternalInput"
    ).ap()
    xt_d = nc.dram_tensor(
        "xt16", [n_groups, PH, 2 * tiles_per_g, PH], F16, kind="ExternalInput"
    ).ap()
    wt_d = nc.dram_tensor("wt16", [PH, 2, 2, PH], F16, kind="ExternalInput").ap()
    bias_d = nc.dram_tensor("bias2", [PH, 2], F32, kind="ExternalInput").ap()
    v_d = nc.dram_tensor("v2", [PH, 2], F16, kind="ExternalInput").ap()
    out = nc.dram_tensor("out", [b_dim, D], F32, kind="ExternalOutput").ap()

    with tile.TileContext(nc) as tc:
        with (
            tc.tile_pool(name="singles", bufs=1) as singles,
            tc.tile_pool(name="xnat", bufs=n_groups) as xnat_pool,
            tc.tile_pool(name="xt", bufs=10) as xt_pool,
            tc.tile_pool(name="usb", bufs=4) as u_pool,
            tc.tile_pool(name="small", bufs=8) as small,
            tc.tile_pool(name="dram", bufs=1, space="DRAM") as dram_pool,
        ):
            # identity first: it unblocks the PE warmup spins
            identity16 = singles.tile([PH, PH], F16)
            from concourse.masks import make_identity
            make_identity(nc, identity16)

            # ---- constants (wt first on sync: it gates mm1-g0) ----
            wt = singles.tile([PH, 2, 2, PH], F16)
            nc.sync.dma_start(out=wt, in_=wt_d)
            bias_sb = singles.tile([PH, 2], F32)
            nc.gpsimd.dma_start(out=bias_sb, in_=bias_d)
            v16 = singles.tile([PH, 2], F16)
            nc.gpsimd.dma_start(out=v16, in_=v_d)

            # transposed scores [q, j] (j = c*64 + b)
            scmat = singles.tile([PH, n_cols], F16)

            # block-diagonal slab for step4: slab[q, j, b'] nonzero only
            # at b' = j%64; zeroed per half on DVE early
            slab = singles.tile([PH, n_cols, b_dim], F16)
            slab_flat = slab.rearrange("p j b -> p (j b)")
            nc.vector.memset(slab_flat[:, 0:4096], 0.0)      # c=0 half
            # (c=1 half is zeroed inside the loop, off the cast path)

            s_sb = singles.tile([b_dim, D], F32)

            xnat_tiles = []

            def emit_xn_load(g, gate=None):
                xn = xnat_pool.tile([PH, tiles_per_g, D], F16, name="xn",
                                    bufs=16)
                if gate is not None:
                    # tiny WAW write: the load waits for it, pacing the
                    # trigger to the compute pipeline
                    nc.vector.tensor_copy(xn[0:1, 0:1, 0:1], gate[0:1, 0:1])
                nc.gpsimd.dma_start(out=xn, in_=xn_d[g])
                xnat_tiles.append(xn)

            def emit_softmax_half(c, gate=None):
                # no max-subtraction: |scores| < ~70, exp fits f32
                e_sb = small.tile([PH, 64], F32, name="esb", bufs=2)
                ssum = small.tile([PH, 1], F32, name="ssum", bufs=2)
                if gate is not None:
                    # WAW pin: stops the scheduler from hoisting this exp
                    # into the mid-kernel tanh stream (ACT is in-order)
                    nc.scalar.copy(e_sb[0:1, 0:1], gate[0:1, 0:1])
                nc.scalar.activation(
                    e_sb, scmat[:, c * 64:(c + 1) * 64], AF.Exp,
                    accum_out=ssum,
                )
                rec = small.tile([PH, 1], F32, name="rec", bufs=2)
                nc.vector.reciprocal(rec, ssum)
                # diagonal: j = c*64+b, flat off = j*64 + b = c*4096 + 65*b
                diag = slab_flat[:, c * 4096:c * 4096 + 63 * 65 + 1:65]
                nc.vector.tensor_scalar_mul(diag, e_sb, rec)

            # ---- PE p-state warmup spins during the DMA fill ----
            with tc.tile_pool(name="warm", bufs=1, space="PSUM") as warm_pool:
                warm_ps = warm_pool.tile([PH, PH], F16)
                for _ in range(24):
                    nc.tensor.transpose(warm_ps, identity16, identity16)

            with (
                tc.tile_pool(name="ups", bufs=3, space="PSUM") as ups_pool,
                tc.tile_pool(name="coll", bufs=1, space="PSUM") as coll_pool,
                tc.tile_pool(name="trp", bufs=1, space="PSUM") as trp_pool,
            ):
                coll = None
                tr_ps = trp_pool.tile([PH, 4, PH], F16)

                xt_tiles = [
                    xt_pool.tile([PH, 2 * tiles_per_g, PH], F16, name="xt",
                                 bufs=n_groups)
                    for _ in range(n_groups)
                ]
                # all xt triggers upfront on sync in consumption order:
                # FIFO queues then deliver groups in the order the PE
                # consumes them, with no cross-engine coupling
                nc.sync.dma_start(
                    out=xt_tiles[0][:, 0:tiles_per_g, :],
                    in_=xt_d[0, :, 0:tiles_per_g, :],
                )
                nc.sync.dma_start(
                    out=xt_tiles[0][:, tiles_per_g:, :],
                    in_=xt_d[0, :, tiles_per_g:, :],
                )
                for g in range(1, n_groups):
                    nc.sync.dma_start(out=xt_tiles[g], in_=xt_d[g])

                for g in range(n_groups):
                    xt = xt_tiles[g]

                    # ---- mm1 + tanh ----
                    u16_cur = []
                    for ec in range(2):
                        u_ps = ups_pool.tile([PH, BTG], F32, name="ups",
                                             bufs=3)
                        for half in range(2):
                            for dc in range(2):
                                m0 = half * 8 + dc
                                nc.tensor.matmul(
                                    u_ps[:, half * 512:(half + 1) * 512],
                                    wt[:, dc, ec, :],
                                    xt[:, m0:m0 + 7:2, :],
                                    start=(dc == 0),
                                    stop=(dc == 1),
                                )
                        u_sb = u_pool.tile([PH, BTG], F16, name="usb", bufs=4)
                        nc.scalar.activation(
                            u_sb, u_ps, AF.Tanh, bias=bias_sb[:, ec:ec + 1]
                        )
                        u16_cur.append(u_sb)

                    if g >= XN_DEFER:
                        emit_xn_load(g - XN_DEFER, gate=u16_cur[0])

                    if g == n_groups - 1:
                        # fill the PE wait on the last tanh with spins
                        for _ in range(6):
                            nc.tensor.transpose(
                                tr_ps[:, 0, :], identity16, identity16
                            )

                    # ---- mm2 into the PSUM collector: piece (g, half)
                    # at partition 32*((g%2)*2 + half) ----
                    if g % 2 == 0:
                        coll = coll_pool.tile([97, 512], F32)
                    for half in range(2):
                        row = 32 * ((g % 2) * 2 + half)
                        for ec in range(2):
                            nc.tensor.matmul(
                                coll[row:row + 1, :],
                                v16[:, ec:ec + 1],
                                u16_cur[ec][:, half * 512:(half + 1) * 512],
                                start=(ec == 0),
                                stop=(ec == 1),
                                tile_position=(0, row),
                            )
                    if g % 2 == 1:
                        # cast 4 pieces at once, then PE-transpose the
                        # four 128-column chunks straight into scmat
                        # columns (no DRAM bounce, no DMA-transpose:
                        # scores stay in SBUF/PSUM end to end). Chunk c
                        # row 32r holds scmat column j = 16p + 4r + c.
                        p = g // 2
                        sc16 = small.tile([97, 512], F16, name="sc16", bufs=2)
                        nc.vector.tensor_copy(sc16, coll)
                        for c in range(4):
                            nc.tensor.transpose(
                                tr_ps[:, c, 0:97],
                                sc16[:, c * PH:(c + 1) * PH],
                                identity16[0:97, 0:97],
                            )
                        nc.vector.tensor_copy(
                            scmat[:, p * 16:(p + 1) * 16]
                            .rearrange("p (r c) -> p c r", c=4),
                            tr_ps[:, :, 0:97:32],
                        )
                    if g == 2:
                        nc.vector.memset(slab_flat[:, 4096:8192], 0.0)

            # ---- tail ----
            with tc.tile_pool(name="s4ps", bufs=1, space="PSUM") as s4_pool:
                s_ps = s4_pool.tile([b_dim, D], F32)
                for g in range(n_groups - XN_DEFER, n_groups):
                    emit_xn_load(g)
                emit_softmax_half(0, gate=u16_cur[0])
                emit_softmax_half(1, gate=u16_cur[1])
                for j in range(n_cols):
                    nc.tensor.matmul(
                        s_ps,
                        slab[:, j, :],
                        xnat_tiles[j // tiles_per_g][:, j % tiles_per_g, :],
                        start=(j == 0),
                        stop=(j == n_cols - 1),
                    )
                nc.vector.tensor_copy(s_sb, s_ps)
                nc.sync.dma_start(out=out, in_=s_sb)

    nc.compile()
    return nc


_prog_cache = {}


def _get_program(b_dim):
    if b_dim not in _prog_cache:
        _prog_cache[b_dim] = build_program(b_dim)
    return _prog_cache[b_dim]


def prep_core_inputs(shard_f32, w, bias, v):
    """Pack one core's [bt, D] f32 shard + weights into device layouts.

    Rows are permuted (c, b, q)-major first: rowP = c*8192 + b*128 + q
    for original row b*256 + c*128 + q.
    """
    bt = shard_f32.shape[0]
    b_dim = bt // T_LOC
    n_groups = bt // BTG
    tiles_per_g = BTG // PH
    h16 = shard_f32.astype(np.float16)
    h16 = np.ascontiguousarray(
        h16.reshape(b_dim, 2, PH, D).transpose(1, 0, 2, 3)
    ).reshape(bt, D)
    # xn16[g, p, i, d] = h16[g*1024 + i*128 + p, d]
    xn16 = np.ascontiguousarray(
        h16.reshape(n_groups, tiles_per_g, PH, D).transpose(0, 2, 1, 3)
    )
    # xt16[g, p, m, q] = h16[g*1024 + (m//2)*128 + q, (m%2)*128 + p]
    hr = h16.reshape(n_groups, tiles_per_g, PH, 2, PH)
    xt16 = np.ascontiguousarray(hr.transpose(0, 4, 1, 3, 2)).reshape(
        n_groups, PH, 2 * tiles_per_g, PH
    )
    # wt16[p, dc, ec, e'] = W[ec*128 + e', dc*128 + p]
    w16 = w.astype(np.float16)
    wt16 = np.ascontiguousarray(
        w16.reshape(2, PH, 2, PH).transpose(3, 2, 0, 1)
    )
    bias2 = np.ascontiguousarray(bias.reshape(2, PH).T).astype(np.float32)
    v2 = np.ascontiguousarray(v.reshape(2, PH).T).astype(np.float16)
    return {
        "xn16": xn16,
        "xt16": xt16,
        "wt16": wt16,
        "bias2": bias2,
        "v2": v2,
    }


def kernel(hidden_states, W_attention, bias_attention, attention_vector):
    from concourse.bass_utils import run_bass_kernel_spmd

    hs = np.asarray(hidden_states, dtype=np.float32)
    w = np.asarray(W_attention, dtype=np.float32)
    bias = np.asarray(bias_attention, dtype=np.float32)
    v = np.asarray(attention_vector, dtype=np.float32)

    nc = _get_program(B)

    in_maps = []
    for core in range(NCORES):
        shard = np.ascontiguousarray(
            hs[:, core * T_LOC:(core + 1) * T_LOC, :]
        ).reshape(BT, D)
        in_maps.append(prep_core_inputs(shard, w, bias, v))

    res = run_bass_kernel_spmd(nc, in_maps, list(range(NCORES)))
    s = np.zeros((B, D), dtype=np.float32)
    for r in res.results:
        s += r["out"]
    return s


# revision 32
# speedup vs baseline: 1.0177x; 1.0039x over previous
"""Trainium2 Bass kernel for nn_AttentionLayer (pooling, dim=0 softmax).

Computation (full shapes B=64, T=2048, D=256):
    u = tanh(hs @ W^T + b)            [B,T,D]
    scores = u @ v                    [B,T]
    a = softmax(scores, axis=0)       (over the batch axis!)
    s[b] = a[b] @ hs[b]               [B,D]

Sharding: sequence-parallel over T across 8 cores (T_loc = 256). The
dim=0 softmax couples samples but not time steps, so each core's
softmax is fully local; only the final weighted sum needs a cross-core
reduction, done on the host (8 x 64KB partials).

The host pre-packs each core's shard into two fp16 layouts (natural
[t, d] for the weighted sum, d-major for the tanh matmul), with rows
permuted (c, b, q)-major (t_loc = c*128 + q, so tile j = c*64 + b is
one sample's half-window and softmax half c needs only groups
[8c, 8c+8)). Device pipeline per group (PE ~97% busy end to end,
fp16 compute / f32 PSUM, rel err ~3e-3):

  1. all 16 xt loads are triggered upfront on the sync queue in
     consumption order (FIFO queues then deliver them in the order the
     PE consumes them); xn loads are paced behind the compute pipeline
     via tiny WAW gate-writes so their traffic stays out of the
     startup window; ~24 identity-transpose spins ramp the PE p-state
     during the initial fill.
  2. PE mm1: z^T[e, bt] = W-chunk @ Xt-chunk (PSUM f32, N=512).
  3. ACT: u = tanh(z + bias), PSUM -> SBUF fp16, per-partition bias.
  4. PE mm2: score pieces [1, 512] = v^T @ u accumulate into a PSUM
     collector [97, 512] at partition rows {0, 32, 64, 96} (the only
     legal output bases for 1-row matmuls). One full-height DVE cast
     per group-pair moves 4 pieces to SBUF fp16 at once.
  5. PE transposes the four 128-col chunks of each cast pair straight
     into scmat[q, j] columns (~47ns each): scores never touch DRAM
     and never ride a DMA, so the softmax input has no completion-
     semaphore latency on the critical path.
  6. softmax over b per half c on contiguous scmat columns; no
     max-subtraction (|scores| < ~70 so exp fits f32 comfortably);
     exp is WAW-gated behind the last tanh so the Tile scheduler
     cannot hoist it into the in-order ACT stream mid-kernel.
     Normalized weights are written into a zero-initialized
     block-diagonal slab via a stride-65 access pattern.
  7. PE step4: 128 matmuls lhsT=slab[:, j, :] (zero-padded a-columns)
     accumulate one [64, 256] PSUM tile in a single accumulation
     group, starting immediately after the last mm2 (the c=0 slab
     half is ready mid-kernel; the c=1 softmax chain completes under
     the first 64 step4 matmuls).
  8. one PSUM->SBUF copy + DMA out s_partial [64, 256] f32; the host
     sums the 8 partials.
"""

import numpy as np

B, T, D = 64, 2048, 256
NCORES = 8
T_LOC = T // NCORES          # 256
BT = B * T_LOC               # 16384 rows per core
BTG = 1024                   # bt rows per pipeline group (2 PSUM banks)
PH = 128                     # partitions
XN_DEFER = 2                 # groups of xn-load deferral


def build_program(b_dim=B):
    import concourse.bacc as bacc
    import concourse.tile as tile
    from concourse import mybir

    F32 = mybir.dt.float32
    F16 = mybir.dt.float16
    AF = mybir.ActivationFunctionType

    assert b_dim == B
    bt = b_dim * T_LOC
    n_groups = bt // BTG          # 16
    tiles_per_g = BTG // PH       # 8
    n_cols = bt // PH             # 128 tiles j = c*64 + b

    nc = bacc.Bacc("TRN2", target_bir_lowering=False, debug=False)

    # Host-prepacked fp16 inputs; rows are permuted (c, b, q)-major:
    #   row = c*8192 + b*128 + q  with  t_loc = c*128 + q
    #   xn16[g, p, i, d]  = hsP[g*1024 + i*128 + p, d]          (natural)
    #   xt16[g, p, m, q]  = hsP[g*1024 + (m//2)*128 + q, (m%2)*128 + p]
    #   wt16[p, dc, ec, e'] = W[ec*128 + e', dc*128 + p]
    xn_d = nc.dram_tensor(
        "xn16", [n_groups, PH, tiles_per_g, D], F16, kind="ExternalInput"
    ).ap()
    xt_d = nc.dram_tensor(
        "xt16", [n_groups, PH, 2 * tiles_per_g, PH], F16, kind="ExternalInput"
    ).ap()
    wt_d = nc.dram_tensor("wt16", [PH, 2, 2, PH], F16, kind="ExternalInput").ap()
    bias_d = nc.dram_tensor("bias2", [PH, 2], F32, kind="ExternalInput").ap()
    v_d = nc.dram_tensor("v2", [PH, 2], F16, kind="ExternalInput").ap()
    out = nc.dram_tensor("out", [b_dim, D], F32, kind="ExternalOutput").ap()

    with tile.TileContext(nc) as tc:
        with (
            tc.tile_pool(name="singles", bufs=1) as singles,
            tc.tile_pool(name="xnat", bufs=n_groups) as xnat_pool,
            tc.tile_pool(name="xt", bufs=10) as xt_pool,
            tc.tile_pool(name="usb", bufs=4) as u_pool,
            tc.tile_pool(name="small", bufs=8) as small,
            tc.tile_pool(name="dram", bufs=1, space="DRAM") as dram_pool,
        ):
            # identity first: it unblocks the PE warmup spins
            identity16 = singles.tile([PH, PH], F16)
            from concourse.masks import make_identity
            make_identity(nc, identity16)

            # ---- constants (wt first on sync: it gates mm1-g0) ----
            wt = singles.tile([PH, 2, 2, PH], F16)
            nc.sync.dma_start(out=wt, in_=wt_d)
            bias_sb = singles.tile([PH, 2], F32)
            nc.gpsimd.dma_start(out=bias_sb, in_=bias_d)
            v16 = singles.tile([PH, 2], F16)
            nc.gpsimd.dma_start(out=v16, in_=v_d)

            # transposed scores [q, j] (j = c*64 + b)
            scmat = singles.tile([PH, n_cols], F16)

            # block-diagonal slab for step4: slab[q, j, b'] nonzero only
            # at b' = j%64; zeroed per half on DVE early
            slab = singles.tile([PH, n_cols, b_dim], F16)
            slab_flat = slab.rearrange("p j b -> p (j b)")
            nc.vector.memset(slab_flat[:, 0:4096], 0.0)      # c=0 half
            # (c=1 half is zeroed inside the loop, off the cast path)

            s_sb = singles.tile([b_dim, D], F32)

            xnat_tiles = []

            def emit_xn_load(g, gate=None):
                xn = xnat_pool.tile([PH, tiles_per_g, D], F16, name="xn",
                                    bufs=16)
                if gate is not None:
                    # tiny WAW write: the load waits for it, pacing the
                    # trigger to the compute pipeline
                    nc.vector.tensor_copy(xn[0:1, 0:1, 0:1], gate[0:1, 0:1])
                nc.gpsimd.dma_start(out=xn, in_=xn_d[g])
                xnat_tiles.append(xn)

            def emit_softmax_half(c, gate=None):
                # no max-subtraction: |scores| < ~70, exp fits f32
                e_sb = small.tile([PH, 64], F32, name="esb", bufs=2)
                ssum = small.tile([PH, 1], F32, name="ssum", bufs=2)
                if gate is not None:
                    # WAW pin: stops the scheduler from hoisting this exp
                    # into the mid-kernel tanh stream (ACT is in-order)
                    nc.scalar.copy(e_sb[0:1, 0:1], gate[0:1, 0:1])
                nc.scalar.activation(
                    e_sb, scmat[:, c * 64:(c + 1) * 64], AF.Exp,
                    accum_out=ssum,
                )
                rec = small.tile([PH, 1], F32, name="rec", bufs=2)
                nc.vector.reciprocal(rec, ssum)
                # diagonal: j = c*64+b, flat off = j*64 + b = c*4096 + 65*b
                diag = slab_flat[:, c * 4096:c * 4096 + 63 * 65 + 1:65]
                nc.vector.tensor_scalar_mul(diag, e_sb, rec)

            # ---- PE p-state warmup spins during the DMA fill ----
            with tc.tile_pool(name="warm", bufs=1, space="PSUM") as warm_pool:
                warm_ps = warm_pool.tile([PH, PH], F16)
                for _ in range(34):
                    nc.tensor.transpose(warm_ps, identity16, identity16)

            with (
                tc.tile_pool(name="ups", bufs=3, space="PSUM") as ups_pool,
                tc.tile_pool(name="coll", bufs=1, space="PSUM") as coll_pool,
                tc.tile_pool(name="trp", bufs=1, space="PSUM") as trp_pool,
            ):
                coll = None
                tr_ps = trp_pool.tile([PH, 4, PH], F16)

                xt_tiles = [
                    xt_pool.tile([PH, 2 * tiles_per_g, PH], F16, name="xt",
                                 bufs=n_groups)
                    for _ in range(n_groups)
                ]
                # all xt triggers upfront on sync in consumption order:
                # FIFO queues then deliver groups in the order the PE
                # consumes them, with no cross-engine coupling
                nc.sync.dma_start(
                    out=xt_tiles[0][:, 0:tiles_per_g, :],
                    in_=xt_d[0, :, 0:tiles_per_g, :],
                )
                nc.sync.dma_start(
                    out=xt_tiles[0][:, tiles_per_g:, :],
                    in_=xt_d[0, :, tiles_per_g:, :],
                )
                for g in range(1, n_groups):
                    nc.sync.dma_start(out=xt_tiles[g], in_=xt_d[g])

                for g in range(n_groups):
                    xt = xt_tiles[g]

                    # ---- mm1 + tanh ----
                    u16_cur = []
                    for ec in range(2):
                        u_ps = ups_pool.tile([PH, BTG], F32, name="ups",
                                             bufs=3)
                        for half in range(2):
                            for dc in range(2):
                                m0 = half * 8 + dc
                                nc.tensor.matmul(
                                    u_ps[:, half * 512:(half + 1) * 512],
                                    wt[:, dc, ec, :],
                                    xt[:, m0:m0 + 7:2, :],
                                    start=(dc == 0),
                                    stop=(dc == 1),
                                )
                        u_sb = u_pool.tile([PH, BTG], F16, name="usb", bufs=4)
                        nc.scalar.activation(
                            u_sb, u_ps, AF.Tanh, bias=bias_sb[:, ec:ec + 1]
                        )
                        u16_cur.append(u_sb)

                    if g >= XN_DEFER:
                        emit_xn_load(g - XN_DEFER, gate=u16_cur[0])

                    if g == n_groups - 1:
                        # fill the PE wait on the last tanh with spins
                        for _ in range(10):
                            nc.tensor.transpose(
                                tr_ps[:, 0, :], identity16, identity16
                            )

                    # ---- mm2 into the PSUM collector: piece (g, half)
                    # at partition 32*((g%2)*2 + half) ----
                    if g % 2 == 0:
                        coll = coll_pool.tile([97, 512], F32)
                    for half in range(2):
                        row = 32 * ((g % 2) * 2 + half)
                        for ec in range(2):
                            nc.tensor.matmul(
                                coll[row:row + 1, :],
                                v16[:, ec:ec + 1],
                                u16_cur[ec][:, half * 512:(half + 1) * 512],
                                start=(ec == 0),
                                stop=(ec == 1),
                                tile_position=(0, row),
                            )
                    if g % 2 == 1:
                        # cast 4 pieces at once, then PE-transpose the
                        # four 128-column chunks straight into scmat
                        # columns (no DRAM bounce, no DMA-transpose:
                        # scores stay in SBUF/PSUM end to end). Chunk c
                        # row 32r holds scmat column j = 16p + 4r + c.
                        p = g // 2
                        sc16 = small.tile([97, 512], F16, name="sc16", bufs=2)
                        nc.vector.tensor_copy(sc16, coll)
                        for c in range(4):
                            nc.tensor.transpose(
                                tr_ps[:, c, 0:97],
                                sc16[:, c * PH:(c + 1) * PH],
                                identity16[0:97, 0:97],
                            )
                        nc.vector.tensor_copy(
                            scmat[:, p * 16:(p + 1) * 16]
                            .rearrange("p (r c) -> p c r", c=4),
                            tr_ps[:, :, 0:97:32],
                        )
                    if g == 2:
                        nc.vector.memset(slab_flat[:, 4096:8192], 0.0)

            # ---- tail ----
            with tc.tile_pool(name="s4ps", bufs=1, space="PSUM") as s4_pool:
                s_ps = s4_pool.tile([b_dim, D], F32)
                for g in range(n_groups - XN_DEFER, n_groups):
                    emit_xn_load(g)
                emit_softmax_half(0, gate=u16_cur[0])
                emit_softmax_half(1, gate=u16_cur[1])
                for j in range(n_cols):
                    nc.tensor.matmul(
                        s_ps,
                        slab[:, j, :],
                        xnat_tiles[j // tiles_per_g][:, j % tiles_per_g, :],
                        start=(j == 0),
                        stop=(j == n_cols - 1),
                    )
                nc.vector.tensor_copy(s_sb, s_ps)
                nc.sync.dma_start(out=out, in_=s_sb)

    nc.compile()
    return nc


_prog_cache = {}


def _get_program(b_dim):
    if b_dim not in _prog_cache:
        _prog_cache[b_dim] = build_program(b_dim)
    return _prog_cache[b_dim]


def prep_core_inputs(shard_f32, w, bias, v):
    """Pack one core's [bt, D] f32 shard + weights into device layouts.

    Rows are permuted (c, b, q)-major first: rowP = c*8192 + b*128 + q
    for original row b*256 + c*128 + q.
    """
    bt = shard_f32.shape[0]
    b_dim = bt // T_LOC
    n_groups = bt // BTG
    tiles_per_g = BTG // PH
    h16 = shard_f32.astype(np.float16)
    h16 = np.ascontiguousarray(
        h16.reshape(b_dim, 2, PH, D).transpose(1, 0, 2, 3)
    ).reshape(bt, D)
    # xn16[g, p, i, d] = h16[g*1024 + i*128 + p, d]
    xn16 = np.ascontiguousarray(
        h16.reshape(n_groups, tiles_per_g, PH, D).transpose(0, 2, 1, 3)
    )
    # xt16[g, p, m, q] = h16[g*1024 + (m//2)*128 + q, (m%2)*128 + p]
    hr = h16.reshape(n_groups, tiles_per_g, PH, 2, PH)
    xt16 = np.ascontiguousarray(hr.transpose(0, 4, 1, 3, 2)).reshape(
        n_groups, PH, 2 * tiles_per_g, PH
    )
    # wt16[p, dc, ec, e'] = W[ec*128 + e', dc*128 + p]
    w16 = w.astype(np.float16)
    wt16 = np.ascontiguousarray(
        w16.reshape(2, PH, 2, PH).transpose(3, 2, 0, 1)
    )
    bias2 = np.ascontiguousarray(bias.reshape(2, PH).T).astype(np.float32)
    v2 = np.ascontiguousarray(v.reshape(2, PH).T).astype(np.float16)
    return {
        "xn16": xn16,
        "xt16": xt16,
        "wt16": wt16,
        "bias2": bias2,
        "v2": v2,
    }


def kernel(hidden_states, W_attention, bias_attention, attention_vector):
    from concourse.bass_utils import run_bass_kernel_spmd

    hs = np.asarray(hidden_states, dtype=np.float32)
    w = np.asarray(W_attention, dtype=np.float32)
    bias = np.asarray(bias_attention, dtype=np.float32)
    v = np.asarray(attention_vector, dtype=np.float32)

    nc = _get_program(B)

    in_maps = []
    for core in range(NCORES):
        shard = np.ascontiguousarray(
            hs[:, core * T_LOC:(core + 1) * T_LOC, :]
        ).reshape(BT, D)
        in_maps.append(prep_core_inputs(shard, w, bias, v))

    res = run_bass_kernel_spmd(nc, in_maps, list(range(NCORES)))
    s = np.zeros((B, D), dtype=np.float32)
    for r in res.results:
        s += r["out"]
    return s


# revision 33
# speedup vs baseline: 1.0571x; 1.0387x over previous
"""Trainium2 Bass kernel for nn_AttentionLayer (pooling, dim=0 softmax).

Computation (full shapes B=64, T=2048, D=256):
    u = tanh(hs @ W^T + b)            [B,T,D]
    scores = u @ v                    [B,T]
    a = softmax(scores, axis=0)       (over the batch axis!)
    s[b] = a[b] @ hs[b]               [B,D]

Sharding: sequence-parallel over T across 8 cores (T_loc = 256). The
dim=0 softmax couples samples but not time steps, so each core's
softmax is fully local; only the final weighted sum needs a cross-core
reduction, done on the host (8 x 64KB partials).

The host pre-packs each core's shard into two fp16 layouts (natural
[t, d] for the weighted sum, d-major for the tanh matmul), with rows
permuted (c, b, q)-major (t_loc = c*128 + q, so tile j = c*64 + b is
one sample's half-window and softmax half c needs only groups
[8c, 8c+8)). Device pipeline per group (PE ~97% busy end to end,
fp16 compute / f32 PSUM, rel err ~3e-3):

  1. all 16 xt loads are triggered upfront on the sync queue in
     consumption order (FIFO queues then deliver them in the order the
     PE consumes them); xn loads are paced behind the compute pipeline
     via tiny WAW gate-writes so their traffic stays out of the
     startup window; ~24 identity-transpose spins ramp the PE p-state
     during the initial fill.
  2. PE mm1: z^T[e, bt] = W-chunk @ Xt-chunk (PSUM f32, N=512).
  3. ACT: u = tanh(z + bias), PSUM -> SBUF fp16, per-partition bias.
  4. PE mm2: score pieces [1, 512] = v^T @ u accumulate into a PSUM
     collector [97, 512] at partition rows {0, 32, 64, 96} (the only
     legal output bases for 1-row matmuls). One full-height DVE cast
     per group-pair moves 4 pieces to SBUF fp16 at once.
  5. PE transposes the four 128-col chunks of each cast pair straight
     into scmat[q, j] columns (~47ns each): scores never touch DRAM
     and never ride a DMA, so the softmax input has no completion-
     semaphore latency on the critical path.
  6. softmax over b per half c on contiguous scmat columns; no
     max-subtraction (|scores| < ~70 so exp fits f32 comfortably);
     exp is WAW-gated behind the last tanh so the Tile scheduler
     cannot hoist it into the in-order ACT stream mid-kernel.
     Normalized weights are written into a zero-initialized
     block-diagonal slab via a stride-65 access pattern.
  7. PE step4: 128 matmuls lhsT=slab[:, j, :] (zero-padded a-columns)
     accumulate one [64, 256] PSUM tile in a single accumulation
     group, starting immediately after the last mm2 (the c=0 slab
     half is ready mid-kernel; the c=1 softmax chain completes under
     the first 64 step4 matmuls).
  8. one PSUM->SBUF copy + DMA out s_partial [64, 256] f32; the host
     sums the 8 partials.
"""

import numpy as np

B, T, D = 64, 2048, 256
NCORES = 8
T_LOC = T // NCORES          # 256
BT = B * T_LOC               # 16384 rows per core
BTG = 1024                   # bt rows per pipeline group (2 PSUM banks)
PH = 128                     # partitions
XN_DEFER = 2                 # groups of xn-load deferral


def build_program(b_dim=B):
    import concourse.bacc as bacc
    import concourse.tile as tile
    from concourse import mybir

    F32 = mybir.dt.float32
    F16 = mybir.dt.float16
    AF = mybir.ActivationFunctionType

    assert b_dim == B
    bt = b_dim * T_LOC
    n_groups = bt // BTG          # 16
    tiles_per_g = BTG // PH       # 8
    n_cols = bt // PH             # 128 tiles j = c*64 + b

    nc = bacc.Bacc("TRN2", target_bir_lowering=False, debug=False)

    # Host-prepacked fp16 inputs; rows are permuted (c, b, q)-major:
    #   row = c*8192 + b*128 + q  with  t_loc = c*128 + q
    #   xn16[g, p, i, d]  = hsP[g*1024 + i*128 + p, d]          (natural)
    #   xt16[g, p, m, q]  = hsP[g*1024 + (m//2)*128 + q, (m%2)*128 + p]
    #   wt16[p, dc, ec, e'] = W[ec*128 + e', dc*128 + p]
    xn_d = nc.dram_tensor(
        "xn16", [n_groups, PH, tiles_per_g, D], F16, kind="ExternalInput"
    ).ap()
    xt_d = nc.dram_tensor(
        "xt16", [n_groups, PH, 2 * tiles_per_g, PH], F16, kind="ExternalInput"
    ).ap()
    wt_d = nc.dram_tensor("wt16", [PH, 2, 2, PH], F16, kind="ExternalInput").ap()
    bias_d = nc.dram_tensor("bias2", [PH, 2], F32, kind="ExternalInput").ap()
    v_d = nc.dram_tensor("v2", [PH, 2], F16, kind="ExternalInput").ap()
    out = nc.dram_tensor("out", [b_dim, D], F32, kind="ExternalOutput").ap()

    with tile.TileContext(nc) as tc:
        with (
            tc.tile_pool(name="singles", bufs=1) as singles,
            tc.tile_pool(name="xnat", bufs=n_groups) as xnat_pool,
            tc.tile_pool(name="xt", bufs=10) as xt_pool,
            tc.tile_pool(name="usb", bufs=4) as u_pool,
            tc.tile_pool(name="small", bufs=8) as small,
            tc.tile_pool(name="dram", bufs=1, space="DRAM") as dram_pool,
        ):
            # identity first: it unblocks the PE warmup spins
            identity16 = singles.tile([PH, PH], F16)
            from concourse.masks import make_identity
            make_identity(nc, identity16)

            # ---- constants (wt first on sync: it gates mm1-g0) ----
            wt = singles.tile([PH, 2, 2, PH], F16)
            nc.sync.dma_start(out=wt, in_=wt_d)
            bias_sb = singles.tile([PH, 2], F32)
            nc.gpsimd.dma_start(out=bias_sb, in_=bias_d)
            v16 = singles.tile([PH, 2], F16)
            nc.gpsimd.dma_start(out=v16, in_=v_d)

            # transposed scores [q, j] (j = c*64 + b)
            scmat = singles.tile([PH, n_cols], F16)

            # block-diagonal slab for step4: slab[q, j, b'] nonzero only
            # at b' = j%64; zeroed per half on DVE early
            slab = singles.tile([PH, n_cols, b_dim], F16)
            slab_flat = slab.rearrange("p j b -> p (j b)")
            nc.vector.memset(slab_flat[:, 0:4096], 0.0)      # c=0 half
            # (c=1 half is zeroed inside the loop, off the cast path)

            s_sb = singles.tile([b_dim, D], F32)

            xnat_tiles = []

            def emit_xn_load(g, gate=None):
                xn = xnat_pool.tile([PH, tiles_per_g, D], F16, name="xn",
                                    bufs=16)
                if gate is not None:
                    # tiny WAW write: the load waits for it, pacing the
                    # trigger to the compute pipeline
                    nc.vector.tensor_copy(xn[0:1, 0:1, 0:1], gate[0:1, 0:1])
                nc.gpsimd.dma_start(out=xn, in_=xn_d[g])
                xnat_tiles.append(xn)

            def emit_softmax_half(c, gate=None):
                # no max-subtraction: |scores| < ~70, exp fits f32
                e_sb = small.tile([PH, 64], F32, name="esb", bufs=2)
                ssum = small.tile([PH, 1], F32, name="ssum", bufs=2)
                if gate is not None:
                    # WAW pin: stops the scheduler from hoisting this exp
                    # into the mid-kernel tanh stream (ACT is in-order)
                    nc.scalar.copy(e_sb[0:1, 0:1], gate[0:1, 0:1])
                nc.scalar.activation(
                    e_sb, scmat[:, c * 64:(c + 1) * 64], AF.Exp,
                    accum_out=ssum,
                )
                rec = small.tile([PH, 1], F32, name="rec", bufs=2)
                nc.vector.reciprocal(rec, ssum)
                # diagonal: j = c*64+b, flat off = j*64 + b = c*4096 + 65*b
                diag = slab_flat[:, c * 4096:c * 4096 + 63 * 65 + 1:65]
                nc.vector.tensor_scalar_mul(diag, e_sb, rec)

            # ---- PE p-state warmup spins during the DMA fill ----
            with tc.tile_pool(name="warm", bufs=1, space="PSUM") as warm_pool:
                warm_ps = warm_pool.tile([PH, PH], F16)
                for _ in range(34):
                    nc.tensor.transpose(warm_ps, identity16, identity16)

            with (
                tc.tile_pool(name="ups", bufs=3, space="PSUM") as ups_pool,
                tc.tile_pool(name="coll", bufs=1, space="PSUM") as coll_pool,
                tc.tile_pool(name="trp", bufs=1, space="PSUM") as trp_pool,
            ):
                coll = None
                tr_ps = trp_pool.tile([PH, 4, PH], F16)

                xt_tiles = [
                    xt_pool.tile([PH, 2 * tiles_per_g, PH], F16, name="xt",
                                 bufs=n_groups)
                    for _ in range(n_groups)
                ]
                # all xt triggers upfront on sync in consumption order:
                # FIFO queues then deliver groups in the order the PE
                # consumes them, with no cross-engine coupling
                nc.sync.dma_start(
                    out=xt_tiles[0][:, 0:tiles_per_g, :],
                    in_=xt_d[0, :, 0:tiles_per_g, :],
                )
                nc.sync.dma_start(
                    out=xt_tiles[0][:, tiles_per_g:, :],
                    in_=xt_d[0, :, tiles_per_g:, :],
                )
                for g in range(1, n_groups):
                    nc.sync.dma_start(out=xt_tiles[g], in_=xt_d[g])

                for g in range(n_groups):
                    xt = xt_tiles[g]

                    # ---- mm1 + tanh ----
                    u16_cur = []
                    for ec in range(2):
                        u_ps = ups_pool.tile([PH, BTG], F32, name="ups",
                                             bufs=3)
                        for half in range(2):
                            for dc in range(2):
                                m0 = half * 8 + dc
                                nc.tensor.matmul(
                                    u_ps[:, half * 512:(half + 1) * 512],
                                    wt[:, dc, ec, :],
                                    xt[:, m0:m0 + 7:2, :],
                                    start=(dc == 0),
                                    stop=(dc == 1),
                                )
                        u_sb = u_pool.tile([PH, BTG], F16, name="usb", bufs=4)
                        nc.scalar.activation(
                            u_sb, u_ps, AF.Tanh, bias=bias_sb[:, ec:ec + 1]
                        )
                        u16_cur.append(u_sb)

                    if g >= XN_DEFER:
                        emit_xn_load(g - XN_DEFER, gate=u16_cur[0])

                    if g == n_groups - 1:
                        # fill the PE wait on the last tanh with spins
                        for _ in range(10):
                            nc.tensor.transpose(
                                tr_ps[:, 0, :], identity16, identity16
                            )

                    # ---- mm2 for a PAIR of groups, emitted together
                    # at odd g: halves the mm1<->mm2 transitions (each
                    # transition exposes ~106ns of stationary load) ----
                    if g % 2 == 0:
                        coll = coll_pool.tile([97, 512], F32)
                        u16_even = u16_cur
                    if g % 2 == 1:
                        for gp, u16 in ((g - 1, u16_even), (g, u16_cur)):
                            for half in range(2):
                                row = 32 * ((gp % 2) * 2 + half)
                                for ec in range(2):
                                    nc.tensor.matmul(
                                        coll[row:row + 1, :],
                                        v16[:, ec:ec + 1],
                                        u16[ec]
                                        [:, half * 512:(half + 1) * 512],
                                        start=(ec == 0),
                                        stop=(ec == 1),
                                        tile_position=(0, row),
                                    )
                        # cast 4 pieces at once, then PE-transpose the
                        # four 128-column chunks straight into scmat
                        # columns (no DRAM bounce, no DMA-transpose:
                        # scores stay in SBUF/PSUM end to end). Chunk c
                        # row 32r holds scmat column j = 16p + 4r + c.
                        p = g // 2
                        sc16 = small.tile([97, 512], F16, name="sc16", bufs=2)
                        nc.vector.tensor_copy(sc16, coll)
                        for c in range(4):
                            nc.tensor.transpose(
                                tr_ps[:, c, 0:97],
                                sc16[:, c * PH:(c + 1) * PH],
                                identity16[0:97, 0:97],
                            )
                        nc.vector.tensor_copy(
                            scmat[:, p * 16:(p + 1) * 16]
                            .rearrange("p (r c) -> p c r", c=4),
                            tr_ps[:, :, 0:97:32],
                        )
                    if g == 2:
                        nc.vector.memset(slab_flat[:, 4096:8192], 0.0)

            # ---- tail ----
            with tc.tile_pool(name="s4ps", bufs=1, space="PSUM") as s4_pool:
                s_ps = s4_pool.tile([b_dim, D], F32)
                for g in range(n_groups - XN_DEFER, n_groups):
                    emit_xn_load(g)
                emit_softmax_half(0, gate=u16_cur[0])
                emit_softmax_half(1, gate=u16_cur[1])
                for j in range(n_cols):
                    nc.tensor.matmul(
                        s_ps,
                        slab[:, j, :],
                        xnat_tiles[j // tiles_per_g][:, j % tiles_per_g, :],
                        start=(j == 0),
                        stop=(j == n_cols - 1),
                    )
                nc.vector.tensor_copy(s_sb, s_ps)
                nc.sync.dma_start(out=out, in_=s_sb)

    nc.compile()
    return nc


_prog_cache = {}


def _get_program(b_dim):
    if b_dim not in _prog_cache:
        _prog_cache[b_dim] = build_program(b_dim)
    return _prog_cache[b_dim]


def prep_core_inputs(shard_f32, w, bias, v):
    """Pack one core's [bt, D] f32 shard + weights into device layouts.

    Rows are permuted (c, b, q)-major first: rowP = c*8192 + b*128 + q
    for original row b*256 + c*128 + q.
    """
    bt = shard_f32.shape[0]
    b_dim = bt // T_LOC
    n_groups = bt // BTG
    tiles_per_g = BTG // PH
    h16 = shard_f32.astype(np.float16)
    h16 = np.ascontiguousarray(
        h16.reshape(b_dim, 2, PH, D).transpose(1, 0, 2, 3)
    ).reshape(bt, D)
    # xn16[g, p, i, d] = h16[g*1024 + i*128 + p, d]
    xn16 = np.ascontiguousarray(
        h16.reshape(n_groups, tiles_per_g, PH, D).transpose(0, 2, 1, 3)
    )
    # xt16[g, p, m, q] = h16[g*1024 + (m//2)*128 + q, (m%2)*128 + p]
    hr = h16.reshape(n_groups, tiles_per_g, PH, 2, PH)
    xt16 = np.ascontiguousarray(hr.transpose(0, 4, 1, 3, 2)).reshape(
        n_groups, PH, 2 * tiles_per_g, PH
    )
    # wt16[p, dc, ec, e'] = W[ec*128 + e', dc*128 + p]
    w16 = w.astype(np.float16)
    wt16 = np.ascontiguousarray(
        w16.reshape(2, PH, 2, PH).transpose(3, 2, 0, 1)
    )
    bias2 = np.ascontiguousarray(bias.reshape(2, PH).T).astype(np.float32)
    v2 = np.ascontiguousarray(v.reshape(2, PH).T).astype(np.float16)
    return {
        "xn16": xn16,
        "xt16": xt16,
        "wt16": wt16,
        "bias2": bias2,
        "v2": v2,
    }


def kernel(hidden_states, W_attention, bias_attention, attention_vector):
    from concourse.bass_utils import run_bass_kernel_spmd

    hs = np.asarray(hidden_states, dtype=np.float32)
    w = np.asarray(W_attention, dtype=np.float32)
    bias = np.asarray(bias_attention, dtype=np.float32)
    v = np.asarray(attention_vector, dtype=np.float32)

    nc = _get_program(B)

    in_maps = []
    for core in range(NCORES):
        shard = np.ascontiguousarray(
            hs[:, core * T_LOC:(core + 1) * T_LOC, :]
        ).reshape(BT, D)
        in_maps.append(prep_core_inputs(shard, w, bias, v))

    res = run_bass_kernel_spmd(nc, in_maps, list(range(NCORES)))
    s = np.zeros((B, D), dtype=np.float32)
    for r in res.results:
        s += r["out"]
    return s
